# revision 1
# baseline (speedup 1.0000x reference)
"""DASR (dense_cnn) Trainium2 kernel — nn_DASR_5712306504091.

Data-parallel over batch B=16 -> 8 NeuronCores x 2 samples. Per core,
activations live in SBUF as [128 partitions = 2 samples x 64 ch, flat
padded 66x66 image + guard zones]. Convs run on the TensorEngine as 9
shifted fp32r matmuls per row-chunk accumulating in PSUM (host-prepacked
block-diagonal [128,128] stationaries). The per-sample dynamic depthwise
conv runs as diagonal-stationary matmuls whose diagonals are built
on-device from the kernel-generating MLP. Epilogues (Prelu/bias/gate/
residual) run on Scalar+Vector engines with interior-only strided writes
so padding stays zero forever.
"""
from contextlib import ExitStack

import numpy as np

import concourse.bacc as bacc
import concourse.bass as bass
import concourse.mybir as mybir
import concourse.tile as tile
from concourse.bass_utils import run_bass_kernel_spmd

F32 = mybir.dt.float32
F32R = mybir.dt.float32r
AF = mybir.ActivationFunctionType
ALU = mybir.AluOpType

G, NB, C, KK, RED, S = 5, 5, 64, 3, 8, 2
MOCO = 256
B, H, W = 16, 64, 64
NCORE = 8
BL = B // NCORE

HP = WP = H + 2          # 66
IMG = HP * WP            # 4356
GUARD = 68
FLAT = IMG + 2 * GUARD   # 4492
TAPS = [dy * WP + dx for dy in (-1, 0, 1) for dx in (-1, 0, 1)]
CHUNK_R0 = [1, 8, 15, 22, 29, 36, 43, 50, 57, 61]
CHUNK_NR = [7, 7, 7, 7, 7, 7, 7, 7, 4, 4]
NDAB = G * NB            # 25
NDA = NDAB * 2           # 50

_PROG_CACHE = {}
DEBUG_TAPS = False


def _bd(m):
    out = np.zeros((128, 128), np.float32)
    out[0:64, 0:64] = m
    out[64:128, 64:128] = m
    return out


def _bdvec(v):
    return np.concatenate([v, v]).astype(np.float32)


def host_prep(x, k_v, head_w, head_b, comp_w, da_kw1, da_kw2, da_cw, da_cb,
              ca_w1, ca_w2, dab_cw, dab_cb, grp_w, grp_b, body_w, body_b,
              up_w, up_b, tail_w, tail_b):
    f = np.float32
    head_st = np.zeros((54, 128), f)
    for ti, (dy, dx) in enumerate([(a, b) for a in (-1, 0, 1) for b in (-1, 0, 1)]):
        for s in range(2):
            for c3 in range(3):
                head_st[ti * 6 + s * 3 + c3, 64 * s:64 * s + 64] = \
                    head_w[:, c3, dy + 1, dx + 1]

    dab_st = np.zeros((128, NDAB, 20, 128), f)
    for g in range(G):
        for n in range(NB):
            gn = g * NB + n
            dab_st[:, gn, 0] = _bd(da_cw[g, n, 0].T)
            dab_st[:, gn, 10] = _bd(da_cw[g, n, 1].T)
            for ti in range(9):
                ky, kx = divmod(ti, 3)
                dab_st[:, gn, 1 + ti] = _bd(dab_cw[g, n, 0][:, :, ky, kx].T)
                dab_st[:, gn, 11 + ti] = _bd(dab_cw[g, n, 1][:, :, ky, kx].T)

    grp_st = np.zeros((128, G, 9, 128), f)
    for g in range(G):
        for ti in range(9):
            ky, kx = divmod(ti, 3)
            grp_st[:, g, ti] = _bd(grp_w[g][:, :, ky, kx].T)
    body_st = np.zeros((128, 9, 128), f)
    for ti in range(9):
        ky, kx = divmod(ti, 3)
        body_st[:, ti] = _bd(body_w[:, :, ky, kx].T)

    up_st = np.zeros((128, 4, 9, 128), f)
    for q in range(4):
        for ti in range(9):
            ky, kx = divmod(ti, 3)
            up_st[:, q, ti] = _bd(up_w[64 * q:64 * q + 64, :, ky, kx].T)

    tail_st4 = np.zeros((4, 9, 128, 24), f)
    for o in range(3):
        for c in range(64):
            for dy in (-1, 0, 1):
                for dx in (-1, 0, 1):
                    wv = tail_w[o, c, dy + 1, dx + 1]
                    for ry in range(2):
                        for rx in range(2):
                            ah, ryp = divmod(ry + dy, 2)
                            aw, rxp = divmod(rx + dx, 2)
                            uc = c * 4 + ryp * 2 + rxp
                            q, i = divmod(uc, 64)
                            ti = (ah + 1) * 3 + (aw + 1)
                            ph = ry * 2 + rx
                            for s in range(2):
                                tail_st4[q, ti, 64 * s + i,
                                         s * 12 + o * 4 + ph] += wv
    tail_st = tail_st4.reshape(36, 128, 24).transpose(1, 0, 2).copy()

    kw1T = np.zeros((64, NDA, 64), f)
    kw2T = np.zeros((64, NDAB, 2, 576), f)
    caw1T = np.zeros((64, NDA, 8), f)
    caw2T = np.zeros((8, NDA, 64), f)
    for g in range(G):
        for n in range(NB):
            gn = g * NB + n
            for j in range(2):
                idx = gn * 2 + j
                kw1T[:, idx] = da_kw1[g, n, j].T
                kw2T[:, gn, j] = da_kw2[g, n, j].T
                caw1T[:, idx] = ca_w1[g, n, j].T
                caw2T[:, idx] = ca_w2[g, n, j].T
    comp_stT = comp_w.T.copy()

    nbias = 112
    bias_all = np.zeros((128, nbias), f)
    bias_all[:, 0] = _bdvec(head_b)
    for g in range(G):
        for n in range(NB):
            for j in range(2):
                idx = (g * NB + n) * 2 + j
                bias_all[:, 1 + idx] = _bdvec(da_cb[g, n, j])
                bias_all[:, 51 + idx] = _bdvec(dab_cb[g, n, j])
    for g in range(G):
        bias_all[:, 101 + g] = _bdvec(grp_b[g])
    bias_all[:, 106] = _bdvec(body_b)
    for q in range(4):
        bias_all[:, 107 + q] = _bdvec(up_b[64 * q:64 * q + 64])
    for s in range(2):
        for o in range(3):
            for ph in range(4):
                bias_all[s * 12 + o * 4 + ph, 111] = tail_b[o]

    mask9 = np.zeros((128, 9, 128), f)
    for t in range(9):
        mask9[:, t, :] = np.eye(128, dtype=f)

    shared = dict(head_st=head_st, dab_st=dab_st, grp_st=grp_st,
                  body_st=body_st, up_st=up_st, tail_st=tail_st,
                  kw1T=kw1T, kw2T=kw2T, caw1T=caw1T, caw2T=caw2T,
                  comp_stT=comp_stT, bias_all=bias_all, mask9=mask9)

    per_core = []
    for core in range(NCORE):
        xs = x[core * BL:(core + 1) * BL]
        xpad = np.zeros((2, 3, HP, WP), f)
        xpad[:, :, 1:65, 1:65] = xs
        xflat = xpad.reshape(2, 3, IMG)
        x9 = np.zeros((54, IMG), f)
        for ti, t in enumerate(TAPS):
            lo, hi = max(0, -t), min(IMG, IMG - t)
            for s in range(2):
                for c3 in range(3):
                    x9[ti * 6 + s * 3 + c3, lo:hi] = xflat[s, c3, lo + t:hi + t]
        kvT = k_v[core * BL:(core + 1) * BL].T.copy()
        per_core.append(dict(x9=x9, kv_in=kvT))
    return shared, per_core


def _chunks():
    for r0, nr in zip(CHUNK_R0, CHUNK_NR):
        yield GUARD + r0 * WP, nr * WP, nr


def _iview(ap_owner, q0, nr):
    v = ap_owner[:, q0:q0 + nr * WP].rearrange("p (a b) -> p a b", a=nr)
    return v[:, :, 1:65]


def build_program():
    nc = bacc.Bacc("TRN2", target_bir_lowering=False)
    d = {}

    def din(name, shape):
        d[name] = nc.dram_tensor(name, shape, F32, kind="ExternalInput")

    din("x9", [54, IMG]); din("kv_in", [256, 2])
    din("head_st", [54, 128]); din("dab_st", [128, NDAB, 20, 128])
    din("grp_st", [128, G, 9, 128]); din("body_st", [128, 9, 128])
    din("up_st", [128, 4, 9, 128]); din("tail_st", [128, 36, 24])
    din("kw1T", [64, NDA, 64]); din("kw2T", [64, NDAB, 2, 576])
    din("caw1T", [64, NDA, 8]); din("caw2T", [8, NDA, 64])
    din("comp_stT", [256, 64]); din("bias_all", [128, 112])
    din("mask9", [128, 9, 128])
    y_out = nc.dram_tensor("y_out", [24, 64, 64], F32, kind="ExternalOutput")
    dbg = {}
    if DEBUG_TAPS:
        for nm, shp in [("dbg_ker", [128, NDA * 9]), ("dbg_att", [128, NDA]),
                        ("dbg_x0", [128, IMG]), ("dbg_dab0", [128, IMG]),
                        ("dbg_t1a", [128, IMG]), ("dbg_t1b", [128, IMG]),
                        ("dbg_t2a", [128, IMG]),
                        ("dbg_grp0", [128, IMG]), ("dbg_body", [128, IMG])]:
            dbg[nm] = nc.dram_tensor(nm, shp, F32, kind="ExternalOutput")
    ker_scr = nc.dram_tensor("ker_scr", [NDA, 2, 576], F32)
    att_scr = nc.dram_tensor("att_scr", [NDA, 2, 64], F32)

    with tile.TileContext(nc) as tc, ExitStack() as ctx:
        acts = ctx.enter_context(tc.tile_pool(name="acts", bufs=1))
        wstp = ctx.enter_context(tc.tile_pool(name="wst", bufs=2))
        once = ctx.enter_context(tc.tile_pool(name="once", bufs=1))
        small = ctx.enter_context(tc.tile_pool(name="small", bufs=1))
        dwsp = ctx.enter_context(tc.tile_pool(name="dws", bufs=2))
        spp = ctx.enter_context(
            tc.tile_pool(name="sps", bufs=2, space=bass.MemorySpace.PSUM))
        pp = ctx.enter_context(
            tc.tile_pool(name="ps", bufs=4, space=bass.MemorySpace.PSUM))

        # ---------------- static tiles ----------------
        x0 = acts.tile([128, FLAT], F32R, tag="x0")
        gin = acts.tile([128, FLAT], F32R, tag="gin")
        res = acts.tile([128, FLAT], F32R, tag="res")
        t1 = acts.tile([128, FLAT], F32R, tag="t1")
        for t in (x0, gin, res, t1):
            nc.vector.memset(t[:].bitcast(F32), 0.0)

        x9t = acts.tile([54, IMG], F32R, tag="t2")   # shares slot with t2
        nc.sync.dma_start(x9t[:], d["x9"][:].bitcast(F32R))
        head_st = small.tile([54, 128], F32R, tag="headst")
        nc.sync.dma_start(head_st[:], d["head_st"][:].bitcast(F32R))
        bias = small.tile([128, 112], F32, tag="bias")
        nc.sync.dma_start(bias[:], d["bias_all"][:])
        mask = small.tile([128, 9, 128], F32, tag="mask")
        nc.sync.dma_start(mask[:], d["mask9"][:])
        ker_all = small.tile([128, NDA * 9], F32, tag="kerall")
        att_all = small.tile([128, NDA], F32, tag="attall")
        caw1T = small.tile([64, NDA, 8], F32, tag="caw1T")
        nc.sync.dma_start(caw1T[:], d["caw1T"][:])
        comp_t = small.tile([128, 2, 64], F32, tag="comp")
        nc.sync.dma_start(comp_t[:],
                          d["comp_stT"].rearrange("(kb p) m -> p kb m", kb=2))
        kv_t = small.tile([128, 2, 2], F32, tag="kvt")
        nc.sync.dma_start(kv_t[:],
                          d["kv_in"].rearrange("(kb p) s -> p kb s", kb=2))

        # ---------------- kv compress ----------------
        kv_ps = spp.tile([64, 2], F32, tag="sp")
        for kb in range(2):
            nc.tensor.matmul(kv_ps[:], comp_t[:, kb, :], kv_t[:, kb, :],
                             start=(kb == 0), stop=(kb == 1))
        kvc = small.tile([64, 2], F32, tag="kvc")
        nc.scalar.activation(kvc[:], kv_ps[:], AF.Prelu, alpha=0.1)

        # ---------------- pregen ----------------
        d1sb = small.tile([64, NDA, 2], F32R, tag="d1sb")
        a1sb = small.tile([8, NDA, 2], F32, tag="a1sb")

        for idx in range(NDA):
            kw1t = wstp.tile([64, 64], F32, tag="kw1t")
            nc.sync.dma_start(kw1t[:], d["kw1T"][:, idx])
            p1 = spp.tile([64, 2], F32, tag="sp")
            nc.tensor.matmul(p1[:], kw1t[:], kvc[:], start=True, stop=True)
            nc.scalar.activation(d1sb[:, idx, :], p1[:], AF.Prelu, alpha=0.1)
        for idx in range(NDA):
            gn, j = divmod(idx, 2)
            kw2t = wstp.tile([64, 576], F32R, tag="kw2t")
            nc.sync.dma_start(kw2t[:], d["kw2T"][:, gn, j].bitcast(F32R))
            kersb = wstp.tile([2, 576], F32, tag="kersb")
            for hh in range(2):
                p2 = spp.tile([2, 288], F32, tag="sp2")
                nc.tensor.matmul(p2[:], d1sb[:, idx, :],
                                 kw2t[:, hh * 288:(hh + 1) * 288],
                                 start=True, stop=True)
                nc.vector.tensor_copy(kersb[:, hh * 288:(hh + 1) * 288], p2[:])
            nc.sync.dma_start(ker_scr[idx], kersb[:])
            nc.sync.dma_start(
                ker_all[:, idx * 9:(idx + 1) * 9],
                ker_scr[idx].rearrange("s (c t) -> (s c) t", t=9))
        for idx in range(NDA):
            p3 = spp.tile([8, 2], F32, tag="sp")
            nc.tensor.matmul(p3[:], caw1T[:, idx, :], kvc[:],
                             start=True, stop=True)
            nc.scalar.activation(a1sb[:, idx, :], p3[:], AF.Prelu, alpha=0.1)
        for idx in range(NDA):
            caw2t = wstp.tile([8, 64], F32, tag="caw2t")
            nc.sync.dma_start(caw2t[:], d["caw2T"][:, idx])
            p4 = spp.tile([2, 64], F32, tag="sp")
            nc.tensor.matmul(p4[:], a1sb[:, idx, :], caw2t[:],
                             start=True, stop=True)
            a2sb = wstp.tile([2, 64], F32, tag="a2sb")
            nc.scalar.activation(a2sb[:], p4[:], AF.Sigmoid)
            nc.sync.dma_start(att_scr[idx], a2sb[:])
            nc.sync.dma_start(
                att_all[:, idx:idx + 1],
                att_scr[idx].rearrange("s (c one) -> (s c) one", one=1))

        # ---------------- head conv ----------------
        for q0, n, nr in _chunks():
            ps = pp.tile([128, n], F32, tag="ps")
            nc.tensor.matmul(ps[:], head_st[:],
                             x9t[:, q0 - GUARD:q0 - GUARD + n],
                             start=True, stop=True)
            ps3 = ps[:].rearrange("p (a b) -> p a b", a=nr)
            nc.scalar.activation(_iview(x0, q0, nr), ps3[:, :, 1:65],
                                 AF.Identity, bias=bias[:, 0:1])
        nc.vector.tensor_copy(res[:, GUARD:GUARD + IMG], x0[:, GUARD:GUARD + IMG])
        nc.vector.tensor_copy(gin[:, GUARD:GUARD + IMG], x0[:, GUARD:GUARD + IMG])
        if DEBUG_TAPS:
            nc.sync.dma_start(dbg["dbg_x0"][:], x0[:, GUARD:GUARD + IMG].bitcast(F32))
            nc.sync.dma_start(dbg["dbg_ker"][:], ker_all[:])
            nc.sync.dma_start(dbg["dbg_att"][:], att_all[:])

        t2 = acts.tile([128, FLAT], F32R, tag="t2")
        nc.vector.memset(t2[:].bitcast(F32), 0.0)

        # ---------------- helpers ----------------
        def dwstat_build(idx):
            dws = dwsp.tile([128, 9, 128], F32R, tag="dws")
            for t in range(9):
                nc.vector.tensor_scalar_mul(
                    dws[:, t, :], mask[:, t, :],
                    ker_all[:, idx * 9 + t:idx * 9 + t + 1])
            return dws

        def dw_conv(src, dst, dws):
            for q0, n, nr in _chunks():
                ps = pp.tile([128, n], F32, tag="ps")
                for ti, t in enumerate(TAPS):
                    nc.tensor.matmul(ps[:], dws[:, ti, :],
                                     src[:, q0 + t:q0 + t + n],
                                     start=(ti == 0), stop=(ti == 8))
                ps3 = ps[:].rearrange("p (a b) -> p a b", a=nr)
                nc.scalar.activation(_iview(dst, q0, nr), ps3[:, :, 1:65],
                                     AF.Prelu, alpha=0.1)

        def conv1x1_gate(src, xres, scratch, dst, st_ap, att_col, cb_col):
            for q0, n, nr in _chunks():
                ps = pp.tile([128, n], F32, tag="ps")
                nc.tensor.matmul(ps[:], st_ap, src[:, q0:q0 + n],
                                 start=True, stop=True)
                ps3 = ps[:].rearrange("p (a b) -> p a b", a=nr)
                vi = _iview(scratch, q0, nr)
                nc.vector.scalar_tensor_tensor(
                    vi, _iview(xres, q0, nr), att_col, ps3[:, :, 1:65],
                    op0=ALU.mult, op1=ALU.add)
                nc.scalar.activation(_iview(dst, q0, nr), vi, AF.Prelu,
                                     bias=cb_col, alpha=0.1)

        def conv3x3(src, dst, st3, bias_col, act=True, res_add=None):
            # When src is dst (in-place conv), the chunk-k epilogue write
            # overlaps chunk-(k+1)'s halo read of the OLD values. Delay each
            # epilogue by one chunk so every halo read precedes the write.
            inplace = src is dst
            pend = None

            def epilogue(ps, q0, nr):
                ps3 = ps[:].rearrange("p (a b) -> p a b", a=nr)
                if act:
                    nc.scalar.activation(_iview(dst, q0, nr), ps3[:, :, 1:65],
                                         AF.Prelu, bias=bias_col, alpha=0.1)
                else:
                    nc.vector.scalar_tensor_tensor(
                        _iview(dst, q0, nr), ps3[:, :, 1:65], bias_col,
                        _iview(res_add, q0, nr), op0=ALU.add, op1=ALU.add)

            for q0, n, nr in _chunks():
                ps = pp.tile([128, n], F32, tag="ps")
                for ti, t in enumerate(TAPS):
                    nc.tensor.matmul(ps[:], st3[:, ti, :],
                                     src[:, q0 + t:q0 + t + n],
                                     start=(ti == 0), stop=(ti == 8))
                if not inplace:
                    epilogue(ps, q0, nr)
                else:
                    if pend is not None:
                        epilogue(*pend)
                    pend = (ps, q0, nr)
            if pend is not None:
                epilogue(*pend)

        # ---------------- body ----------------
        for g in range(G):
            for n_ in range(NB):
                gn = g * NB + n_
                wst = wstp.tile([128, 20, 128], F32R, tag="wst")
                nc.sync.dma_start(wst[:], d["dab_st"][:, gn].bitcast(F32R))
                ia, ib = gn * 2, gn * 2 + 1
                dwsA = dwstat_build(ia)
                dw_conv(res, t1, dwsA)
                if DEBUG_TAPS and gn == 0:
                    nc.sync.dma_start(dbg["dbg_t1a"][:],
                                      t1[:, GUARD:GUARD + IMG].bitcast(F32))
                conv1x1_gate(t1, res, t2, t1, wst[:, 0, :],
                             att_all[:, ia:ia + 1], bias[:, 1 + ia:2 + ia])
                if DEBUG_TAPS and gn == 0:
                    nc.sync.dma_start(dbg["dbg_t1b"][:],
                                      t1[:, GUARD:GUARD + IMG].bitcast(F32))
                conv3x3(t1, t2, wst[:, 1:10, :], bias[:, 51 + ia:52 + ia])
                if DEBUG_TAPS and gn == 0:
                    nc.sync.dma_start(dbg["dbg_t2a"][:],
                                      t2[:, GUARD:GUARD + IMG].bitcast(F32))
                dwsB = dwstat_build(ib)
                dw_conv(t2, t1, dwsB)
                conv1x1_gate(t1, t2, t2, t1, wst[:, 10, :],
                             att_all[:, ib:ib + 1], bias[:, 1 + ib:2 + ib])
                conv3x3(t1, res, wst[:, 11:20, :], bias[:, 51 + ib:52 + ib],
                        act=False, res_add=res)
                if DEBUG_TAPS and gn == 0:
                    nc.sync.dma_start(dbg["dbg_dab0"][:],
                                      res[:, GUARD:GUARD + IMG].bitcast(F32))
            gst = wstp.tile([128, 9, 128], F32R, tag="gst")
            nc.sync.dma_start(gst[:], d["grp_st"][:, g].bitcast(F32R))
            conv3x3(res, res, gst[:, :, :], bias[:, 101 + g:102 + g],
                    act=False, res_add=gin)
            if DEBUG_TAPS and g == 0:
                nc.sync.dma_start(dbg["dbg_grp0"][:],
                                  res[:, GUARD:GUARD + IMG].bitcast(F32))
            if g < G - 1:
                nc.vector.tensor_copy(gin[:, GUARD:GUARD + IMG],
                                      res[:, GUARD:GUARD + IMG])

        bst = wstp.tile([128, 9, 128], F32R, tag="gst")
        nc.sync.dma_start(bst[:], d["body_st"][:].bitcast(F32R))
        conv3x3(res, res, bst[:, :, :], bias[:, 106:107],
                act=False, res_add=x0)
        if DEBUG_TAPS:
            nc.sync.dma_start(dbg["dbg_body"][:],
                              res[:, GUARD:GUARD + IMG].bitcast(F32))

        # ---------------- upsampler ----------------
        ust = once.tile([128, 4, 9, 128], F32R, tag="ust")
        nc.sync.dma_start(ust[:], d["up_st"][:].bitcast(F32R))
        uts = []
        for q, tg in enumerate(("gin", "t1", "t2", "x0")):
            ut = acts.tile([128, FLAT], F32R, tag=tg)
            nc.vector.memset(ut[:].bitcast(F32), 0.0)
            uts.append(ut)
            for q0, n, nr in _chunks():
                ps = pp.tile([128, n], F32, tag="ps")
                for ti, t in enumerate(TAPS):
                    nc.tensor.matmul(ps[:], ust[:, q, ti, :],
                                     res[:, q0 + t:q0 + t + n],
                                     start=(ti == 0), stop=(ti == 8))
                ps3 = ps[:].rearrange("p (a b) -> p a b", a=nr)
                nc.scalar.activation(_iview(ut, q0, nr), ps3[:, :, 1:65],
                                     AF.Identity, bias=bias[:, 107 + q:108 + q])

        # ---------------- fused tail + pixel shuffle ----------------
        tst = once.tile([128, 36, 24], F32R, tag="tst")
        nc.sync.dma_start(tst[:], d["tail_st"][:].bitcast(F32R))
        otail = acts.tile([24, FLAT], F32, tag="res")
        for q0, n, nr in _chunks():
            ps = pp.tile([24, n], F32, tag="ps")
            k = 0
            for q in range(4):
                for ti, t in enumerate(TAPS):
                    nc.tensor.matmul(ps[:], tst[:, q * 9 + ti, :],
                                     uts[q][:, q0 + t:q0 + t + n],
                                     start=(k == 0), stop=(k == 35))
                    k += 1
            ps3 = ps[:].rearrange("p (a b) -> p a b", a=nr)
            o3 = otail[:, q0:q0 + nr * WP].rearrange("p (a b) -> p a b", a=nr)
            nc.scalar.activation(o3[:, :, 1:65], ps3[:, :, 1:65],
                                 AF.Identity, bias=bias[0:24, 111:112])

        # ---------------- output DMA (phase-major; host de-shuffles) ----------------
        osrc = otail[0:24, GUARD + WP:GUARD + WP + 64 * WP] \
            .rearrange("p (a b) -> p a b", a=64)[:, :, 1:65]
        nc.sync.dma_start(y_out[:], osrc)

    nc.compile()
    return nc


def kernel(**inputs):
    inputs = {k: np.asarray(v, dtype=np.float32) for k, v in inputs.items()}
    shared, per_core = host_prep(**inputs)

    if "nc" not in _PROG_CACHE:
        _PROG_CACHE["nc"] = build_program()
    nc = _PROG_CACHE["nc"]

    in_maps = [{**shared, **pc} for pc in per_core]
    last_err = None
    for _attempt in range(3):
        try:
            res = run_bass_kernel_spmd(nc, in_maps, core_ids=list(range(NCORE)))
            break
        except Exception as e:
            last_err = e
    else:
        raise last_err

    out = np.zeros((B, 3, 2 * H, 2 * W), np.float32)
    for core in range(NCORE):
        ph = res.results[core]["y_out"].reshape(2, 3, 2, 2, 64, 64)
        out[core * BL:(core + 1) * BL] = \
            ph.transpose(0, 1, 4, 2, 5, 3).reshape(2, 3, 128, 128)
    return out



# revision 3
# speedup vs baseline: 2.2636x; 2.2636x over previous
"""DASR (dense_cnn) Trainium2 kernel — nn_DASR_5712306504091, v2.

Data-parallel over batch B=16 -> 8 NeuronCores x 2 samples. Per core,
activations live in SBUF as [128 partitions = 2 samples x 64 ch, flat
padded 66x66 image + guard zones]. The 3x3 convs inside the DAB blocks
and the per-sample dynamic depthwise convs run as fp8 DoubleRow matmuls
(two taps fused per matmul as k-tiles, 0.5 cycles/row); stream-critical
convs (head, 1x1, group/body tails, upsampler, tail) stay fp32r. The
kernel-generating MLP (ker/att from k_v) is computed on host. Epilogues
run on Scalar+Vector engines with interior-only strided writes so
padding stays zero forever; fp8 shadow copies of the stream are
maintained by GpSimd/Vector converts.
"""
from contextlib import ExitStack

import numpy as np
import ml_dtypes

import concourse.bacc as bacc
import concourse.bass as bass
import concourse.mybir as mybir
import concourse.tile as tile
from concourse.ap import AP
from concourse.bass_utils import run_bass_kernel_spmd

F32 = mybir.dt.float32
F32R = mybir.dt.float32r
F8 = mybir.dt.float8e4
NP8 = ml_dtypes.float8_e4m3
AF = mybir.ActivationFunctionType
ALU = mybir.AluOpType
DR = mybir.MatmulPerfMode.DoubleRow

G, NB, C, KK, RED, S = 5, 5, 64, 3, 8, 2
MOCO = 256
B, H, W = 16, 64, 64
NCORE = 8
BL = B // NCORE

HP = WP = H + 2          # 66
IMG = HP * WP            # 4356
GUARD = 68
FLAT = IMG + 2 * GUARD   # 4492
TAPS = [dy * WP + dx for dy in (-1, 0, 1) for dx in (-1, 0, 1)]
PAIR_T0 = [TAPS[0], TAPS[2], TAPS[4], TAPS[6], TAPS[8]]
PAIR_D = [TAPS[1] - TAPS[0], TAPS[3] - TAPS[2], TAPS[5] - TAPS[4],
          TAPS[7] - TAPS[6], 1]
CHUNK_R0 = [1, 8, 15, 22, 29, 36, 43, 50, 57, 61]
CHUNK_NR = [7, 7, 7, 7, 7, 7, 7, 7, 4, 4]
NDAB = G * NB            # 25
NDA = NDAB * 2           # 50

_PROG_CACHE = {}


def _lrelu(x):
    return np.where(x > 0, x, 0.1 * x).astype(np.float32)


def _bd(m):
    out = np.zeros((128, 128), np.float32)
    out[0:64, 0:64] = m
    out[64:128, 64:128] = m
    return out


def _bdvec(v):
    return np.concatenate([v, v]).astype(np.float32)


def host_prep(x, k_v, head_w, head_b, comp_w, da_kw1, da_kw2, da_cw, da_cb,
              ca_w1, ca_w2, dab_cw, dab_cb, grp_w, grp_b, body_w, body_b,
              up_w, up_b, tail_w, tail_b):
    f = np.float32
    # head conv stationaries: [6 in-part = (sample, ch3), 9 taps, 128 out]
    head_st9 = np.zeros((6, 9, 128), f)
    for ti in range(9):
        ky, kx = divmod(ti, 3)
        for s in range(2):
            for c3 in range(3):
                head_st9[s * 3 + c3, ti, 64 * s:64 * s + 64] = \
                    head_w[:, c3, ky, kx]

    # DAB weights: 1x1 convs fp32, 3x3 convs fp8 pre-paired for DoubleRow
    dab1_st = np.zeros((128, NDAB, 2, 128), f)
    dab3q = np.zeros((128, NDAB, 2, 5, 2, 128), NP8)
    for g in range(G):
        for n in range(NB):
            gn = g * NB + n
            for j in range(2):
                dab1_st[:, gn, j] = _bd(da_cw[g, n, j].T)
                for p in range(5):
                    for i in range(2):
                        ti = 2 * p + i
                        if ti > 8:
                            continue
                        ky, kx = divmod(ti, 3)
                        dab3q[:, gn, j, p, i] = _bd(
                            dab_cw[g, n, j][:, :, ky, kx].T).astype(NP8)

    grp_st = np.zeros((128, G, 9, 128), f)
    for g in range(G):
        for ti in range(9):
            ky, kx = divmod(ti, 3)
            grp_st[:, g, ti] = _bd(grp_w[g][:, :, ky, kx].T)
    body_st = np.zeros((128, 9, 128), f)
    for ti in range(9):
        ky, kx = divmod(ti, 3)
        body_st[:, ti] = _bd(body_w[:, :, ky, kx].T)

    up_st = np.zeros((128, 4, 9, 128), f)
    for q in range(4):
        for ti in range(9):
            ky, kx = divmod(ti, 3)
            up_st[:, q, ti] = _bd(up_w[64 * q:64 * q + 64, :, ky, kx].T)

    tail_st4 = np.zeros((4, 9, 128, 24), f)
    for o in range(3):
        for c in range(64):
            for dy in (-1, 0, 1):
                for dx in (-1, 0, 1):
                    wv = tail_w[o, c, dy + 1, dx + 1]
                    for ry in range(2):
                        for rx in range(2):
                            ah, ryp = divmod(ry + dy, 2)
                            aw, rxp = divmod(rx + dx, 2)
                            uc = c * 4 + ryp * 2 + rxp
                            q, i = divmod(uc, 64)
                            ti = (ah + 1) * 3 + (aw + 1)
                            ph = ry * 2 + rx
                            for s in range(2):
                                tail_st4[q, ti, 64 * s + i,
                                         s * 12 + o * 4 + ph] += wv
    tail_st = tail_st4.reshape(36, 128, 24).transpose(1, 0, 2).copy()

    nbias = 112
    bias_all = np.zeros((128, nbias), f)
    bias_all[:, 0] = _bdvec(head_b)
    for g in range(G):
        for n in range(NB):
            for j in range(2):
                idx = (g * NB + n) * 2 + j
                bias_all[:, 1 + idx] = _bdvec(da_cb[g, n, j])
                bias_all[:, 51 + idx] = _bdvec(dab_cb[g, n, j])
    for g in range(G):
        bias_all[:, 101 + g] = _bdvec(grp_b[g])
    bias_all[:, 106] = _bdvec(body_b)
    for q in range(4):
        bias_all[:, 107 + q] = _bdvec(up_b[64 * q:64 * q + 64])
    for s in range(2):
        for o in range(3):
            for ph in range(4):
                bias_all[s * 12 + o * 4 + ph, 111] = tail_b[o]

    mask10 = np.zeros((128, 10, 128), f)
    for t in range(9):
        mask10[:, t, :] = np.eye(128, dtype=f)

    # ---- host-side kernel-generating MLP (depends only on k_v) ----
    kv = _lrelu(k_v @ comp_w.T)                       # [B, 64]
    ker_b = np.zeros((B, NDA, 576), f)
    att_b = np.zeros((B, NDA, 64), f)
    for g in range(G):
        for n in range(NB):
            for j in range(2):
                idx = (g * NB + n) * 2 + j
                ker_b[:, idx] = _lrelu(kv @ da_kw1[g, n, j].T) @ \
                    da_kw2[g, n, j].T
                a1 = _lrelu(kv @ ca_w1[g, n, j].T) @ ca_w2[g, n, j].T
                att_b[:, idx] = 1.0 / (1.0 + np.exp(-a1))

    shared = dict(head_st9=head_st9, dab1_st=dab1_st, dab3q=dab3q,
                  grp_st=grp_st, body_st=body_st, up_st=up_st,
                  tail_st=tail_st, bias_all=bias_all, mask10=mask10)

    per_core = []
    for core in range(NCORE):
        xs = x[core * BL:(core + 1) * BL]
        xpad = np.zeros((2, 3, HP, WP), f)
        xpad[:, :, 1:65, 1:65] = xs
        x6 = np.zeros((6, FLAT), f)
        x6[:, GUARD:GUARD + IMG] = xpad.reshape(6, IMG)
        ker_all = np.zeros((128, NDA * 10), f)
        att_all = np.zeros((128, NDA), f)
        for s in range(2):
            b = core * BL + s
            kb = ker_b[b].reshape(NDA, 64, 9)
            for idx in range(NDA):
                ker_all[64 * s:64 * s + 64,
                        idx * 10:idx * 10 + 9] = kb[idx]
                att_all[64 * s:64 * s + 64, idx] = att_b[b, idx]
        per_core.append(dict(x6=x6, ker_all=ker_all, att_all=att_all))
    return shared, per_core


def _chunks():
    for r0, nr in zip(CHUNK_R0, CHUNK_NR):
        yield GUARD + r0 * WP, nr * WP, nr


def _iview(ap_owner, q0, nr):
    v = ap_owner[:, q0:q0 + nr * WP].rearrange("p (a b) -> p a b", a=nr)
    return v[:, :, 1:65]


def build_program():
    nc = bacc.Bacc("TRN2", target_bir_lowering=False)
    d = {}

    def din(name, shape, dt=F32):
        d[name] = nc.dram_tensor(name, shape, dt, kind="ExternalInput")

    din("x6", [6, FLAT])
    din("ker_all", [128, NDA * 10])
    din("att_all", [128, NDA])
    din("head_st9", [6, 9, 128])
    din("dab1_st", [128, NDAB, 2, 128])
    din("dab3q", [128, NDAB, 2, 5, 2, 128], F8)
    din("grp_st", [128, G, 9, 128])
    din("body_st", [128, 9, 128])
    din("up_st", [128, 4, 9, 128])
    din("tail_st", [128, 36, 24])
    din("bias_all", [128, 112])
    din("mask10", [128, 10, 128])
    y_out = nc.dram_tensor("y_out", [24, 64, 64], F32, kind="ExternalOutput")

    with tile.TileContext(nc) as tc, ExitStack() as ctx:
        acts = ctx.enter_context(tc.tile_pool(name="acts", bufs=1))
        wstp = ctx.enter_context(tc.tile_pool(name="wst", bufs=2))
        once = ctx.enter_context(tc.tile_pool(name="once", bufs=1))
        small = ctx.enter_context(tc.tile_pool(name="small", bufs=1))
        dwsp = ctx.enter_context(tc.tile_pool(name="dws", bufs=3))
        pp = ctx.enter_context(
            tc.tile_pool(name="ps", bufs=6, space=bass.MemorySpace.PSUM))

        # ---------------- static tiles ----------------
        x0 = acts.tile([128, FLAT], F32R, tag="x0")
        gin = acts.tile([128, FLAT], F32R, tag="gin")
        res = acts.tile([128, FLAT], F32R, tag="res")
        t1 = acts.tile([128, FLAT], F32R, tag="t1")
        t2 = acts.tile([128, FLAT], F32R, tag="t2")
        resq = acts.tile([128, FLAT], F8, tag="resq")
        t1q = acts.tile([128, FLAT], F8, tag="t1q")
        t2q = acts.tile([128, FLAT], F8, tag="t2q")
        nc.vector.memset(x0[:].bitcast(F32), 0.0)
        nc.vector.memset(t1[:].bitcast(F32), 0.0)
        nc.vector.memset(t2[:].bitcast(F32), 0.0)
        nc.vector.memset(gin[:].bitcast(F32), 0.0)
        nc.vector.memset(res[:].bitcast(F32), 0.0)
        nc.gpsimd.memset(resq[:], 0.0)
        nc.gpsimd.memset(t1q[:], 0.0)
        nc.gpsimd.memset(t2q[:], 0.0)

        x6t = small.tile([6, FLAT], F32R, tag="x6")
        nc.sync.dma_start(x6t[:], d["x6"][:].bitcast(F32R))
        hst = small.tile([6, 9, 128], F32R, tag="hst")
        nc.sync.dma_start(hst[:], d["head_st9"][:].bitcast(F32R))
        bias = small.tile([128, 112], F32, tag="bias")
        nc.sync.dma_start(bias[:], d["bias_all"][:])
        maskt = small.tile([128, 10, 128], F32, tag="mask")
        nc.sync.dma_start(maskt[:], d["mask10"][:])
        kat = small.tile([128, NDA * 10], F32, tag="kerall")
        nc.sync.dma_start(kat[:], d["ker_all"][:])
        att = small.tile([128, NDA], F32, tag="attall")
        nc.sync.dma_start(att[:], d["att_all"][:])

        # ---------------- helpers ----------------
        def dws_build(gn):
            """One GpSimd op building both halves' 10 fp8 diag k-tiles."""
            dws = dwsp.tile([128, 2, 10, 128], F8, tag="dws")
            ia = gn * 2
            m_b = maskt[:].unsqueeze(1).broadcast_to([128, 2, 10, 128])
            k_b = kat[:, ia * 10:(ia + 2) * 10] \
                .rearrange("p (a b) -> p a b", a=2) \
                .unsqueeze(3).broadcast_to([128, 2, 10, 128])
            nc.gpsimd.tensor_tensor(dws[:], m_b, k_b, ALU.mult)
            return dws

        def dr_rhs(srcq, q0, p, n):
            base = srcq[:, 0:1]
            return AP(base.tensor, q0 + PAIR_T0[p],
                      [[FLAT, 128], [PAIR_D[p], 2], [1, n]])

        def dw_conv(srcq, dst, dws_h):
            for q0, n, nr in _chunks():
                ps = pp.tile([128, n], F32, tag="ps")
                for p in range(5):
                    nc.tensor.matmul(ps[:], dws_h[:, 2 * p:2 * p + 2, :],
                                     dr_rhs(srcq, q0, p, n),
                                     start=(p == 0), stop=(p == 4),
                                     perf_mode=DR)
                ps3 = ps[:].rearrange("p (a b) -> p a b", a=nr)
                nc.scalar.activation(_iview(dst, q0, nr), ps3[:, :, 1:65],
                                     AF.Prelu, alpha=0.1)

        def conv1x1_gate(src, xres, scratch, dstq, st_ap, att_col, cb_col):
            for q0, n, nr in _chunks():
                ps = pp.tile([128, n], F32, tag="ps")
                nc.tensor.matmul(ps[:], st_ap, src[:, q0:q0 + n],
                                 start=True, stop=True)
                ps3 = ps[:].rearrange("p (a b) -> p a b", a=nr)
                vi = _iview(scratch, q0, nr)
                nc.vector.scalar_tensor_tensor(
                    vi, _iview(xres, q0, nr), att_col, ps3[:, :, 1:65],
                    op0=ALU.mult, op1=ALU.add)
                nc.scalar.activation(_iview(dstq, q0, nr), vi, AF.Prelu,
                                     bias=cb_col, alpha=0.1)

        def conv3x3_dr(srcq, w_h, dst, bias_col, act=True, res_add=None,
                       conv_dst=None, conv_eng=None):
            for q0, n, nr in _chunks():
                ps = pp.tile([128, n], F32, tag="ps")
                for p in range(5):
                    nc.tensor.matmul(ps[:], w_h[:, p], dr_rhs(srcq, q0, p, n),
                                     start=(p == 0), stop=(p == 4),
                                     perf_mode=DR)
                ps3 = ps[:].rearrange("p (a b) -> p a b", a=nr)
                if act:
                    nc.scalar.activation(_iview(dst, q0, nr), ps3[:, :, 1:65],
                                         AF.Prelu, bias=bias_col, alpha=0.1)
                else:
                    nc.vector.scalar_tensor_tensor(
                        _iview(dst, q0, nr), ps3[:, :, 1:65], bias_col,
                        _iview(res_add, q0, nr), op0=ALU.add, op1=ALU.add)
                if conv_dst is not None:
                    conv_eng.tensor_copy(_iview(conv_dst, q0, nr),
                                         _iview(dst, q0, nr))

        def conv3x3_f32(src, dst, st3, bias_col, res_add, conv_dst=None,
                        conv_eng=None):
            # in-place fp32r conv (src is dst): delay each epilogue by one
            # chunk so every halo read precedes the write.
            inplace = src is dst
            pend = None

            def epilogue(ps, q0, nr):
                ps3 = ps[:].rearrange("p (a b) -> p a b", a=nr)
                nc.vector.scalar_tensor_tensor(
                    _iview(dst, q0, nr), ps3[:, :, 1:65], bias_col,
                    _iview(res_add, q0, nr), op0=ALU.add, op1=ALU.add)
                if conv_dst is not None:
                    conv_eng.tensor_copy(_iview(conv_dst, q0, nr),
                                         _iview(dst, q0, nr))

            for q0, n, nr in _chunks():
                ps = pp.tile([128, n], F32, tag="ps")
                for ti, t in enumerate(TAPS):
                    nc.tensor.matmul(ps[:], st3[:, ti, :],
                                     src[:, q0 + t:q0 + t + n],
                                     start=(ti == 0), stop=(ti == 8))
                if not inplace:
                    epilogue(ps, q0, nr)
                else:
                    if pend is not None:
                        epilogue(*pend)
                    pend = (ps, q0, nr)
            if pend is not None:
                epilogue(*pend)

        # ---------------- head conv (9 taps from 6 partitions) ----------
        dws_tiles = {0: dws_build(0)}
        for q0, n, nr in _chunks():
            ps = pp.tile([128, n], F32, tag="ps")
            for ti, t in enumerate(TAPS):
                nc.tensor.matmul(ps[:], hst[:, ti, :],
                                 x6t[:, q0 + t:q0 + t + n],
                                 start=(ti == 0), stop=(ti == 8))
            ps3 = ps[:].rearrange("p (a b) -> p a b", a=nr)
            nc.scalar.activation(_iview(x0, q0, nr), ps3[:, :, 1:65],
                                 AF.Identity, bias=bias[:, 0:1])
            nc.gpsimd.tensor_copy(_iview(resq, q0, nr), _iview(x0, q0, nr))
        nc.vector.tensor_copy(res[:, GUARD:GUARD + IMG],
                              x0[:, GUARD:GUARD + IMG])
        nc.vector.tensor_copy(gin[:, GUARD:GUARD + IMG],
                              x0[:, GUARD:GUARD + IMG])

        # ---------------- body ----------------
        for g in range(G):
            for n_ in range(NB):
                gn = g * NB + n_
                if gn + 1 < NDAB:
                    dws_tiles[gn + 1] = dws_build(gn + 1)
                wst1 = wstp.tile([128, 2, 128], F32R, tag="wst1")
                nc.sync.dma_start(wst1[:], d["dab1_st"][:, gn].bitcast(F32R))
                wst3 = wstp.tile([128, 2, 5, 2, 128], F8, tag="wst3")
                nc.sync.dma_start(wst3[:], d["dab3q"][:, gn])
                dws = dws_tiles.pop(gn)
                ia, ib = gn * 2, gn * 2 + 1
                dw_conv(resq, t1, dws[:, 0])
                conv1x1_gate(t1, res, t2, t1q, wst1[:, 0, :],
                             att[:, ia:ia + 1], bias[:, 1 + ia:2 + ia])
                conv3x3_dr(t1q, wst3[:, 0], t2, bias[:, 51 + ia:52 + ia],
                           act=True, conv_dst=t2q, conv_eng=nc.vector)
                dw_conv(t2q, t1, dws[:, 1])
                conv1x1_gate(t1, t2, t2, t1q, wst1[:, 1, :],
                             att[:, ib:ib + 1], bias[:, 1 + ib:2 + ib])
                conv3x3_dr(t1q, wst3[:, 1], res, bias[:, 51 + ib:52 + ib],
                           act=False, res_add=res,
                           conv_dst=resq, conv_eng=nc.gpsimd)
            gst = wstp.tile([128, 9, 128], F32R, tag="gst")
            nc.sync.dma_start(gst[:], d["grp_st"][:, g].bitcast(F32R))
            conv3x3_f32(res, res, gst[:, :, :], bias[:, 101 + g:102 + g],
                        res_add=gin, conv_dst=resq, conv_eng=nc.gpsimd)
            if g < G - 1:
                nc.vector.tensor_copy(gin[:, GUARD:GUARD + IMG],
                                      res[:, GUARD:GUARD + IMG])

        bst = wstp.tile([128, 9, 128], F32R, tag="gst")
        nc.sync.dma_start(bst[:], d["body_st"][:].bitcast(F32R))
        conv3x3_f32(res, res, bst[:, :, :], bias[:, 106:107], res_add=x0)

        # ---------------- upsampler ----------------
        ust = once.tile([128, 4, 9, 128], F32R, tag="ust")
        nc.sync.dma_start(ust[:], d["up_st"][:].bitcast(F32R))
        uts = []
        for q, tg in enumerate(("gin", "t1", "t2", "x0")):
            ut = acts.tile([128, FLAT], F32R, tag=tg)
            uts.append(ut)
            for q0, n, nr in _chunks():
                ps = pp.tile([128, n], F32, tag="ps")
                for ti, t in enumerate(TAPS):
                    nc.tensor.matmul(ps[:], ust[:, q, ti, :],
                                     res[:, q0 + t:q0 + t + n],
                                     start=(ti == 0), stop=(ti == 8))
                ps3 = ps[:].rearrange("p (a b) -> p a b", a=nr)
                nc.scalar.activation(_iview(ut, q0, nr), ps3[:, :, 1:65],
                                     AF.Identity, bias=bias[:, 107 + q:108 + q])

        # ---------------- fused tail + pixel shuffle ----------------
        tst = once.tile([128, 36, 24], F32R, tag="tst")
        nc.sync.dma_start(tst[:], d["tail_st"][:].bitcast(F32R))
        otail = acts.tile([24, FLAT], F32, tag="res")
        for q0, n, nr in _chunks():
            ps = pp.tile([24, n], F32, tag="ps")
            k = 0
            for q in range(4):
                for ti, t in enumerate(TAPS):
                    nc.tensor.matmul(ps[:], tst[:, q * 9 + ti, :],
                                     uts[q][:, q0 + t:q0 + t + n],
                                     start=(k == 0), stop=(k == 35))
                    k += 1
            ps3 = ps[:].rearrange("p (a b) -> p a b", a=nr)
            o3 = otail[:, q0:q0 + nr * WP].rearrange("p (a b) -> p a b", a=nr)
            nc.scalar.activation(o3[:, :, 1:65], ps3[:, :, 1:65],
                                 AF.Identity, bias=bias[0:24, 111:112])

        # ---------------- output DMA (phase-major; host de-shuffles) -----
        osrc = otail[0:24, GUARD + WP:GUARD + WP + 64 * WP] \
            .rearrange("p (a b) -> p a b", a=64)[:, :, 1:65]
        nc.sync.dma_start(y_out[:], osrc)

    nc.compile()
    return nc


def kernel(**inputs):
    inputs = {k: np.asarray(v, dtype=np.float32) for k, v in inputs.items()}
    shared, per_core = host_prep(**inputs)

    if "nc" not in _PROG_CACHE:
        _PROG_CACHE["nc"] = build_program()
    nc = _PROG_CACHE["nc"]

    in_maps = [{**shared, **pc} for pc in per_core]
    last_err = None
    for _attempt in range(3):
        try:
            res = run_bass_kernel_spmd(nc, in_maps, core_ids=list(range(NCORE)))
            break
        except Exception as e:
            last_err = e
    else:
        raise last_err

    out = np.zeros((B, 3, 2 * H, 2 * W), np.float32)
    for core in range(NCORE):
        ph = res.results[core]["y_out"].reshape(2, 3, 2, 2, 64, 64)
        out[core * BL:(core + 1) * BL] = \
            ph.transpose(0, 1, 4, 2, 5, 3).reshape(2, 3, 128, 128)
    return out
